# revision 1
# baseline (speedup 1.0000x reference)
"""DGI (Deep Graph Infomax) Trainium2 kernel.

Strategy (8 NeuronCores, one shared SPMD program):
  - Nodes sharded by destination: core c owns dst nodes [c*N/8, (c+1)*N/8).
  - xw = x @ W computed replicated on every core in fp16 (DMA-transpose
    loads + PE matmuls), written to per-core DRAM.
  - GCN aggregation: edges (incl. self-loops, symmetric norm precomputed on
    host) are sorted by (dst tile, src<32768), padded to 128-edge tiles with
    (idx=0, w=0, dstl=-1).  Each 128-edge tile: dma_gather of xw rows
    (int16 indices; hi half gathered from an offset AP), weighted one-hot
    S_Tw built on DVE (is_equal vs iota, scaled by norm), PE matmul
    S_Tw.T @ gathered accumulated into the dst tile's PSUM.
  - PReLU(agg + b) -> z tiles; z1 kept in SBUF, z2 streamed.
  - summary = sigmoid(mean(z1)): DVE tree column-sum + ones-matmul,
    1KB AllReduce across the 8 cores, sigmoid on ACT.
  - wsum = disc_W @ summary via PE (host passes disc_W.T); broadcast via
    K=1 matmul; pos/neg = z . wsum via fused tensor_tensor_reduce.
  - Per-core [128, DT] outputs; host unshards/concatenates.
"""

import os

import numpy as np

_P = 128
_LO = 32768
_C = 8


def _build_streams(sidx, ed, ew, C, NS, DT):
    """Build per-core gather/weight/dstl streams with a shared tile structure.

    sidx: source index per edge (already permuted for the corrupted pass)
    ed:   destination node per edge
    ew:   edge weight (symmetric norm) per edge
    Returns (idx_sbuf [C,128,n_et*8] i16, w_sbuf [C,128,n_et] f32,
             dl_sbuf [C,128,n_et] f16, Tmax [DT,2] int, off_tiles [DT,2] int,
             n_et)
    """
    core = ed // NS
    ldst = ed - core * NS
    dt = ldst // _P
    dstl = ldst % _P
    cls = (sidx >= _LO).astype(np.int64)

    gid = (core * DT + dt) * 2 + cls
    NG = C * DT * 2
    cnt = np.bincount(gid, minlength=NG).reshape(C, DT, 2)
    T = -(-cnt // _P)
    Tmax = T.max(axis=0)  # shared structure across cores
    flat = Tmax.reshape(-1)
    off_tiles = np.concatenate([[0], np.cumsum(flat)[:-1]]).reshape(DT, 2)
    n_et = int(flat.sum())

    order = np.argsort(gid, kind="stable")
    sorted_gid = gid[order]
    g_starts = np.concatenate(
        [[0], np.cumsum(np.bincount(sorted_gid, minlength=NG))[:-1]]
    )
    rank = np.arange(order.size) - g_starts[sorted_gid]
    g_dt = (sorted_gid // 2) % DT
    g_cls = sorted_gid % 2
    pos = off_tiles[g_dt, g_cls] * _P + rank
    core_s = sorted_gid // (DT * 2)

    L = n_et * _P
    idx16 = np.zeros((C, L), np.int16)
    wv = np.zeros((C, L), np.float32)
    dl = np.full((C, L), -1.0, np.float16)
    sidx_s = sidx[order]
    idx16[core_s, pos] = (sidx_s - g_cls * _LO).astype(np.int16)
    wv[core_s, pos] = ew[order]
    dl[core_s, pos] = dstl[order].astype(np.float16)

    idx_w = idx16.reshape(C, L // 16, 16).transpose(0, 2, 1)
    idx_sbuf = np.ascontiguousarray(np.tile(idx_w, (1, 8, 1)))
    w_sbuf = np.ascontiguousarray(wv.reshape(C, n_et, _P).transpose(0, 2, 1))
    dl_sbuf = np.ascontiguousarray(dl.reshape(C, n_et, _P).transpose(0, 2, 1))
    return idx_sbuf, w_sbuf, dl_sbuf, Tmax, off_tiles, n_et


def kernel(x, W, b, a, disc_W, edge_index, perm):
    import bass_rust
    import concourse.bacc as bacc
    import concourse.mybir as mybir
    import concourse.tile as tile
    from concourse.bass_utils import run_bass_kernel_spmd

    x = np.asarray(x)
    W = np.asarray(W)
    b = np.asarray(b, np.float32)
    a = np.asarray(a, np.float32)
    disc_W = np.asarray(disc_W, np.float32)
    ei = np.asarray(edge_index, np.int64)
    perm_np = np.asarray(perm, np.int64)

    N, F = x.shape
    H = W.shape[1]
    C = _C
    NS = N // C
    DT = -(-NS // _P)
    LAST = NS - (DT - 1) * _P  # valid rows of the last dst tile
    f16 = mybir.dt.float16
    f32 = mybir.dt.float32

    # ---- host preprocessing -------------------------------------------
    src = ei[0]
    dst = ei[1]
    deg = (np.bincount(dst, minlength=N) + 1.0).astype(np.float32)
    dinv = (1.0 / np.sqrt(deg)).astype(np.float32)
    loops = np.arange(N, dtype=np.int64)
    es = np.concatenate([src, loops])
    ed = np.concatenate([dst, loops])
    ew = dinv[es] * dinv[ed]
    es2 = perm_np[es]

    i1, w1, d1, T1, O1, n_et1 = _build_streams(es, ed, ew, C, NS, DT)
    i2, w2, d2, T2, O2, n_et2 = _build_streams(es2, ed, ew, C, NS, DT)

    x_f16 = np.ascontiguousarray(x.astype(np.float16))
    W_f16 = np.ascontiguousarray(W.astype(np.float16))
    dwT = np.ascontiguousarray(disc_W.T.astype(np.float32))
    iota_np = np.tile(np.arange(_P, dtype=np.float16)[None, :], (_P, 1))

    # ---- device program -----------------------------------------------
    nc = bacc.Bacc("TRN2", target_bir_lowering=False, debug=False, num_devices=C)

    t_x = nc.dram_tensor("x16", [N, F], f16, kind="ExternalInput")
    t_W = nc.dram_tensor("w16", [F, H], f16, kind="ExternalInput")
    t_b = nc.dram_tensor("bvec", [H], f32, kind="ExternalInput")
    t_a = nc.dram_tensor("avec", [1], f32, kind="ExternalInput")
    t_dwT = nc.dram_tensor("dwT", [H, H], f32, kind="ExternalInput")
    t_iota = nc.dram_tensor("iota", [_P, _P], f16, kind="ExternalInput")
    t_ident = nc.dram_tensor("ident_in", [_P, _P], f32, kind="ExternalInput")
    t_i1 = nc.dram_tensor("idx1", [_P, n_et1 * 8], mybir.dt.int16, kind="ExternalInput")
    t_w1 = nc.dram_tensor("wgt1", [_P, n_et1], f32, kind="ExternalInput")
    t_d1 = nc.dram_tensor("dstl1", [_P, n_et1], f16, kind="ExternalInput")
    t_i2 = nc.dram_tensor("idx2", [_P, n_et2 * 8], mybir.dt.int16, kind="ExternalInput")
    t_w2 = nc.dram_tensor("wgt2", [_P, n_et2], f32, kind="ExternalInput")
    t_d2 = nc.dram_tensor("dstl2", [_P, n_et2], f16, kind="ExternalInput")

    t_pos = nc.dram_tensor("pos_out", [_P, DT], f32, kind="ExternalOutput")
    t_neg = nc.dram_tensor("neg_out", [_P, DT], f32, kind="ExternalOutput")

    t_xw = nc.dram_tensor("xw", [N, H], f16)
    t_ar_in = nc.dram_tensor("ar_in", [H], f32)
    t_ar_out = nc.dram_tensor("ar_out", [H], f32, addr_space="Shared")

    CHUNK = 512  # phase-1 node rows per transposed load
    STAGE = int(os.environ.get("KERNEL_STAGE", "4"))

    with tile.TileContext(nc) as tc:
        import contextlib

        ctx = contextlib.ExitStack()
        consts = ctx.enter_context(tc.tile_pool(name="consts", bufs=1))
        ph1 = ctx.enter_context(tc.tile_pool(name="ph1", bufs=3))
        ph1ps = ctx.enter_context(tc.tile_pool(name="ph1ps", bufs=2, space="PSUM"))
        glo = ctx.enter_context(tc.tile_pool(name="glo", bufs=2))
        ghi = ctx.enter_context(tc.tile_pool(name="ghi", bufs=2))
        stp = ctx.enter_context(tc.tile_pool(name="stp", bufs=4))
        aggps = ctx.enter_context(tc.tile_pool(name="aggps", bufs=3, space="PSUM"))
        misc = ctx.enter_context(tc.tile_pool(name="misc", bufs=2))
        miscps = ctx.enter_context(tc.tile_pool(name="miscps", bufs=1, space="PSUM"))

        # ---- constants ----
        W0 = consts.tile([_P, H], f16, tag="W0")
        W1 = consts.tile([_P, H], f16, tag="W1")
        nc.sync.dma_start(W0[:], t_W[0:_P, :])
        nc.sync.dma_start(W1[:], t_W[_P : 2 * _P, :])
        iota_t = consts.tile([_P, _P], f16, tag="iota")
        nc.sync.dma_start(iota_t[:], t_iota[:])
        b_sb = consts.tile([1, H], f32, tag="b_sb")
        nc.sync.dma_start(b_sb[:], t_b[None, :])
        a_sb = consts.tile([1, 1], f32, tag="a_sb")
        nc.sync.dma_start(a_sb[:], t_a[None, :])
        dwT0 = consts.tile([_P, H], f32, tag="dwT0")
        dwT1 = consts.tile([_P, H], f32, tag="dwT1")
        nc.sync.dma_start(dwT0[:], t_dwT[0:_P, :])
        nc.sync.dma_start(dwT1[:], t_dwT[_P : 2 * _P, :])
        ones_row = consts.tile([1, _P], f32, tag="ones_row")
        nc.vector.memset(ones_row[:], 1.0)
        ones_col = consts.tile([_P, 1], f32, tag="ones_col")
        nc.vector.memset(ones_col[:], 1.0)

        # broadcasts via K=1 matmul
        bb_ps = miscps.tile([_P, H], f32, tag="mps")
        nc.tensor.matmul(bb_ps[:], ones_row[:], b_sb[:], start=True, stop=True)
        b_bc = consts.tile([_P, H], f32, tag="b_bc")
        nc.vector.tensor_copy(b_bc[:], bb_ps[:])
        ab_ps = miscps.tile([_P, 1], f32, tag="mps")
        nc.tensor.matmul(ab_ps[:], ones_row[:], a_sb[:], start=True, stop=True)
        a_bc = consts.tile([_P, 1], f32, tag="a_bc")
        nc.vector.tensor_copy(a_bc[:], ab_ps[:])

        # ---- stream loads ----
        i1_sb = consts.tile([_P, n_et1 * 8], mybir.dt.int16, tag="i1")
        w1_sb = consts.tile([_P, n_et1], f32, tag="w1")
        d1_sb = consts.tile([_P, n_et1], f16, tag="d1")
        nc.sync.dma_start(i1_sb[:], t_i1[:])
        nc.sync.dma_start(w1_sb[:], t_w1[:])
        nc.sync.dma_start(d1_sb[:], t_d1[:])
        i2_sb = consts.tile([_P, n_et2 * 8], mybir.dt.int16, tag="i2")
        w2_sb = consts.tile([_P, n_et2], f32, tag="w2")
        d2_sb = consts.tile([_P, n_et2], f16, tag="d2")
        nc.sync.dma_start(i2_sb[:], t_i2[:])
        nc.sync.dma_start(w2_sb[:], t_w2[:])
        nc.sync.dma_start(d2_sb[:], t_d2[:])

        # ---- phase 1: xw = x @ W (replicated) ----
        for r0 in range(0, N, CHUNK):
            rows = min(CHUNK, N - r0)
            xT0 = ph1.tile([_P, CHUNK], f16, tag="xT0")
            xT1 = ph1.tile([_P, CHUNK], f16, tag="xT1")
            nc.sync.dma_start_transpose(xT0[:, :rows], t_x[r0 : r0 + rows, 0:_P])
            nc.sync.dma_start_transpose(
                xT1[:, :rows], t_x[r0 : r0 + rows, _P : 2 * _P]
            )
            for o in range(0, rows, _P):
                m = min(_P, rows - o)
                ps = ph1ps.tile([_P, H], f32, tag="ph1ps")
                nc.tensor.matmul(
                    ps[:m, :], xT0[:, o : o + m], W0[:], start=True, stop=False
                )
                nc.tensor.matmul(
                    ps[:m, :], xT1[:, o : o + m], W1[:], start=False, stop=True
                )
                xw_sb = ph1.tile([_P, H], f16, tag="xw_sb")
                nc.any.tensor_copy(xw_sb[:m, :], ps[:m, :])
                nc.sync.dma_start(t_xw[r0 + o : r0 + o + m, :], xw_sb[:m, :])

        xw_lo = t_xw[0:_LO, :] if N > _LO else t_xw[:, :]
        xw_hi = t_xw[_LO:N, :] if N > _LO else None

        # ---- aggregation passes ----
        zbuf = consts.tile([_P, DT * H], f32, tag="zbuf")  # z1 persists

        def agg_pass(idx_sb, w_sb, dl_sb, Tm, Ot, z_consumer):
            for dti in range(DT):
                Tl, Th = int(Tm[dti, 0]), int(Tm[dti, 1])
                gl = gh = None
                if Tl:
                    o = int(Ot[dti, 0])
                    gl = glo.tile([_P, max_Tl, H], f16, tag="gl")
                    nc.gpsimd.dma_gather(
                        gl[:, :Tl, :],
                        xw_lo,
                        idx_sb[:, 8 * o : 8 * (o + Tl)],
                        Tl * _P,
                        Tl * _P,
                        H,
                        single_packet=(Tl * _P <= 1024),
                    )
                if Th:
                    o = int(Ot[dti, 1])
                    gh = ghi.tile([_P, max_Th, H], f16, tag="gh")
                    nc.gpsimd.dma_gather(
                        gh[:, :Th, :],
                        xw_hi,
                        idx_sb[:, 8 * o : 8 * (o + Th)],
                        Th * _P,
                        Th * _P,
                        H,
                        single_packet=(Th * _P <= 1024),
                    )
                ps = aggps.tile([_P, H], f32, tag="aggps")
                n_mm = Tl + Th
                k = 0
                for cls_i, (Tn, g, o0) in enumerate(
                    [(Tl, gl, int(Ot[dti, 0])), (Th, gh, int(Ot[dti, 1]))]
                ):
                    for j in range(Tn):
                        t = o0 + j
                        eq = stp.tile([_P, _P], f16, tag="eq")
                        nc.vector.tensor_tensor(
                            eq[:],
                            dl_sb[:, t : t + 1].to_broadcast([_P, _P]),
                            iota_t[:],
                            mybir.AluOpType.is_equal,
                        )
                        stw = stp.tile([_P, _P], f16, tag="stw")
                        nc.vector.tensor_scalar(
                            stw[:],
                            eq[:],
                            w_sb[:, t : t + 1],
                            None,
                            mybir.AluOpType.mult,
                        )
                        nc.tensor.matmul(
                            ps[:],
                            stw[:],
                            g[:, j, :],
                            start=(k == 0),
                            stop=(k == n_mm - 1),
                        )
                        k += 1
                z_consumer(dti, ps)

        # z1 consumer: bias + PReLU into persistent zbuf
        def z1_consume(dti, ps):
            zs = zbuf[:, dti * H : (dti + 1) * H]
            nc.vector.tensor_tensor(zs, ps[:], b_bc[:], mybir.AluOpType.add)
            t1 = misc.tile([_P, H], f32, tag="t1")
            nc.vector.tensor_scalar(
                t1[:], zs, 0.0, a_bc[:, 0:1],
                mybir.AluOpType.min, mybir.AluOpType.mult,
            )
            t2 = misc.tile([_P, H], f32, tag="t2")
            nc.vector.tensor_scalar(t2[:], zs, 0.0, None, mybir.AluOpType.max)
            nc.vector.tensor_tensor(zs, t1[:], t2[:], mybir.AluOpType.add)

        max_Tl = max(int(T1[:, 0].max()), int(T2[:, 0].max()), 1)
        max_Th = max(int(T1[:, 1].max()), int(T2[:, 1].max()), 1)

        pos_acc = consts.tile([_P, DT], f32, tag="pos_acc")
        neg_acc = consts.tile([_P, DT], f32, tag="neg_acc")
        nc.vector.memset(pos_acc[:], 0.0)
        nc.vector.memset(neg_acc[:], 0.0)

        if STAGE >= 2:
            agg_pass(i1_sb, w1_sb, d1_sb, T1, O1, z1_consume)

        # ---- summary: column sum of z1 over all nodes ----
        if STAGE >= 3:
            cacc = misc.tile([_P, H], f32, tag="cacc")
            nc.vector.tensor_copy(cacc[:], zbuf[:, 0:H])
            for dti in range(1, DT):
                rows = LAST if dti == DT - 1 else _P
                nc.vector.tensor_tensor(
                    cacc[:rows, :],
                    cacc[:rows, :],
                    zbuf[:rows, dti * H : (dti + 1) * H],
                    mybir.AluOpType.add,
                )
            cs_ps = miscps.tile([1, H], f32, tag="mps")
            nc.tensor.matmul(cs_ps[:], ones_col[:], cacc[:], start=True, stop=True)
            cs_sb = misc.tile([1, H], f32, tag="cs_sb")
            nc.vector.tensor_copy(cs_sb[:], cs_ps[:])
            nc.sync.dma_start(t_ar_in[None, :], cs_sb[:])
            nc.gpsimd.collective_compute(
                "AllReduce",
                mybir.AluOpType.add,
                replica_groups=[list(range(C))],
                ins=[t_ar_in[:]],
                outs=[t_ar_out[:]],
            )
            sums_sb = misc.tile([1, H], f32, tag="sums_sb")
            nc.sync.dma_start(sums_sb[:], t_ar_out[None, :])
            summ_sb = misc.tile([1, H], f32, tag="summ_sb")
            nc.scalar.activation(
                summ_sb[:], sums_sb[:], mybir.ActivationFunctionType.Sigmoid,
                scale=1.0 / N,
            )

            # ---- wsum = disc_W @ summary ----
            ident = consts.tile([_P, _P], f32, tag="ident")
            nc.sync.dma_start(ident[:], t_ident[:])
            sT = misc.tile([_P, 2], f32, tag="sT")
            for c_i in range(2):
                tp = miscps.tile([_P, _P], f32, tag="mps")
                nc.tensor.transpose(
                    tp[:, 0:1],
                    summ_sb[0:1, c_i * _P : (c_i + 1) * _P],
                    ident[0:1, 0:1],
                )
                nc.vector.tensor_copy(sT[:, c_i : c_i + 1], tp[:, 0:1])
            ws_ps = miscps.tile([1, H], f32, tag="mps")
            nc.tensor.matmul(ws_ps[:], sT[:, 0:1], dwT0[:], start=True, stop=False)
            nc.tensor.matmul(ws_ps[:], sT[:, 1:2], dwT1[:], start=False, stop=True)
            ws_sb = misc.tile([1, H], f32, tag="ws_sb")
            nc.vector.tensor_copy(ws_sb[:], ws_ps[:])
            wb_ps = miscps.tile([_P, H], f32, tag="mps")
            nc.tensor.matmul(wb_ps[:], ones_row[:], ws_sb[:], start=True, stop=True)
            wsum_bc = consts.tile([_P, H], f32, tag="wsum_bc")
            nc.vector.tensor_copy(wsum_bc[:], wb_ps[:])

        scratch = misc.tile([_P, H], f32, tag="scratch")

        # ---- z2 pass with inline neg dot ----
        def z2_consume(dti, ps):
            zt = misc.tile([_P, H], f32, tag="zt")
            nc.vector.tensor_tensor(zt[:], ps[:], b_bc[:], mybir.AluOpType.add)
            t1 = misc.tile([_P, H], f32, tag="t1")
            nc.vector.tensor_scalar(
                t1[:], zt[:], 0.0, a_bc[:, 0:1],
                mybir.AluOpType.min, mybir.AluOpType.mult,
            )
            t2 = misc.tile([_P, H], f32, tag="t2")
            nc.vector.tensor_scalar(t2[:], zt[:], 0.0, None, mybir.AluOpType.max)
            nc.vector.tensor_tensor(zt[:], t1[:], t2[:], mybir.AluOpType.add)
            nc.vector.tensor_tensor(
                scratch[:], zt[:], wsum_bc[:], mybir.AluOpType.mult
            )
            nc.vector.reduce_sum(
                neg_acc[:, dti : dti + 1], scratch[:], bass_rust.AxisListType.X
            )

        if STAGE >= 4:
            agg_pass(i2_sb, w2_sb, d2_sb, T2, O2, z2_consume)

            # ---- pos dots from persistent z1 ----
            for dti in range(DT):
                nc.vector.tensor_tensor(
                    scratch[:], zbuf[:, dti * H : (dti + 1) * H], wsum_bc[:],
                    mybir.AluOpType.mult,
                )
                nc.vector.reduce_sum(
                    pos_acc[:, dti : dti + 1], scratch[:], bass_rust.AxisListType.X
                )

        nc.sync.dma_start(t_pos[:], pos_acc[:])
        nc.sync.dma_start(t_neg[:], neg_acc[:])
        ctx.close()

    nc.compile()

    in_maps = []
    for c in range(C):
        in_maps.append(
            {
                "x16": x_f16,
                "w16": W_f16,
                "bvec": b,
                "avec": a,
                "dwT": dwT,
                "iota": iota_np,
                "ident_in": np.eye(_P, dtype=np.float32),
                "idx1": i1[c],
                "wgt1": w1[c],
                "dstl1": d1[c],
                "idx2": i2[c],
                "wgt2": w2[c],
                "dstl2": d2[c],
            }
        )

    if os.environ.get("KERNEL_SIM", "0") == "1":
        from concourse import bass_interp

        sim = bass_interp.MultiCoreSim(nc, C)
        for c in range(C):
            for k, v in in_maps[c].items():
                sim.cores[c].tensor(k)[:] = v
        sim.simulate()
        results = [
            {
                "pos_out": np.array(sim.cores[c].tensor("pos_out")),
                "neg_out": np.array(sim.cores[c].tensor("neg_out")),
            }
            for c in range(C)
        ]
    else:
        trace = os.environ.get("KERNEL_TRACE", "0") == "1"
        kw = {}
        if trace:
            kw["trace"] = True
        res = run_bass_kernel_spmd(nc, in_maps, core_ids=list(range(C)), **kw)
        kernel.last_result = res
        results = res.results

    pos = np.zeros(N, np.float32)
    neg = np.zeros(N, np.float32)
    for c in range(C):
        pos[c * NS : (c + 1) * NS] = results[c]["pos_out"].T.reshape(-1)[:NS]
        neg[c * NS : (c + 1) * NS] = results[c]["neg_out"].T.reshape(-1)[:NS]
    return pos, neg



# revision 10
# speedup vs baseline: 1.9385x; 1.9385x over previous
"""DGI (Deep Graph Infomax) Trainium2 kernel — fused dual-pass design.

Strategy (8 NeuronCores, one shared SPMD program):
  - Nodes sharded by destination: core c owns dst nodes [c*N/8, (c+1)*N/8).
  - Key identity: z2 = encode(x[perm]) needs xw[perm[src]] per edge, and
    xw[perm[n]] = (x[perm] @ W)[n].  So the device materializes fused rows
    xw_cat[n] = [ (x@W)[n] | (x_perm@W)[n] ]  (512 f16 = 1KB per row) and a
    SINGLE dma_gather per (dst tile, lo/hi class) feeds BOTH passes — the
    edge indices, weights and one-hot structure are identical.  This halves
    the GPSIMD SWDGE descriptor-generation cost (the baseline bottleneck).
  - Phase 1: host passes x^T and (x[perm])^T pre-transposed f16 (no DMA
    transposes); 4 matmuls per 128 nodes accumulate [xw | xw_p] in one
    PSUM bank, copied f16 and stored as 1KB DRAM rows.
  - Aggregation per dst tile: gather lo/hi rows, build the weighted one-hot
    S_Tw for ALL the tile's edge-tiles in 2 batched DVE ops (3D broadcast
    APs), then one 512-wide matmul per edge-tile accumulating pos/neg
    halves into a single PSUM bank.  PSUM -> zcat (f16, persistent).
  - Tail: batched bias+PReLU over zcat, strided-reduce column sums of z1,
    1KB AllReduce, sigmoid, wsum = disc_W @ summary, then in-place
    zcat *= [wsum|wsum] and one strided reduce -> [128, DT*2] pos/neg.
"""

import os

import numpy as np

_P = 128
_LO = 32768
_C = 8


def _build_streams(sidx, ed, ew, C, NS, DT):
    """Per-core gather/weight/dstl streams with a shared tile structure.

    sidx: source index per edge; ed: destination node; ew: edge weight.
    Returns (idx_sbuf [C,128,n_et*8] i16, w_sbuf [C,128,n_et] f16,
             dl_sbuf [C,128,n_et] f16, Tmax [DT,2] int, off_tiles [DT,2] int,
             n_et)
    """
    core = ed // NS
    ldst = ed - core * NS
    dt = ldst // _P
    dstl = ldst % _P
    cls = (sidx >= _LO).astype(np.int64)

    gid = (core * DT + dt) * 2 + cls
    NG = C * DT * 2
    cnt = np.bincount(gid, minlength=NG).reshape(C, DT, 2)
    T = -(-cnt // _P)
    Tmax = T.max(axis=0)  # shared structure across cores
    flat = Tmax.reshape(-1)
    off_tiles = np.concatenate([[0], np.cumsum(flat)[:-1]]).reshape(DT, 2)
    n_et = int(flat.sum())

    order = np.argsort(gid, kind="stable")
    sorted_gid = gid[order]
    g_starts = np.concatenate(
        [[0], np.cumsum(np.bincount(sorted_gid, minlength=NG))[:-1]]
    )
    rank = np.arange(order.size) - g_starts[sorted_gid]
    g_dt = (sorted_gid // 2) % DT
    g_cls = sorted_gid % 2
    pos = off_tiles[g_dt, g_cls] * _P + rank
    core_s = sorted_gid // (DT * 2)

    L = n_et * _P
    idx16 = np.zeros((C, L), np.int16)
    wv = np.zeros((C, L), np.float16)
    dl = np.full((C, L), -1.0, np.float16)
    sidx_s = sidx[order]
    idx16[core_s, pos] = (sidx_s - g_cls * _LO).astype(np.int16)
    wv[core_s, pos] = ew[order].astype(np.float16)
    dl[core_s, pos] = dstl[order].astype(np.float16)

    idx_w = idx16.reshape(C, L // 16, 16).transpose(0, 2, 1)
    idx_sbuf = np.ascontiguousarray(np.tile(idx_w, (1, 8, 1)))
    w_sbuf = np.ascontiguousarray(wv.reshape(C, n_et, _P).transpose(0, 2, 1))
    dl_sbuf = np.ascontiguousarray(dl.reshape(C, n_et, _P).transpose(0, 2, 1))
    return idx_sbuf, w_sbuf, dl_sbuf, Tmax, off_tiles, n_et


def kernel(x, W, b, a, disc_W, edge_index, perm):
    import bass_rust
    import concourse.bacc as bacc
    import concourse.mybir as mybir
    import concourse.tile as tile
    from concourse.bass_utils import run_bass_kernel_spmd

    x = np.asarray(x)
    W = np.asarray(W)
    b = np.asarray(b, np.float32)
    a = np.asarray(a, np.float32)
    disc_W = np.asarray(disc_W, np.float32)
    ei = np.asarray(edge_index, np.int64)
    perm_np = np.asarray(perm, np.int64)

    N, F = x.shape
    H = W.shape[1]
    H2 = 2 * H
    C = _C
    NS = N // C
    DT = -(-NS // _P)
    LAST = NS - (DT - 1) * _P  # valid rows of the last dst tile
    f16 = mybir.dt.float16
    f32 = mybir.dt.float32

    # ---- host preprocessing -------------------------------------------
    src = ei[0]
    dst = ei[1]
    deg = (np.bincount(dst, minlength=N) + 1.0).astype(np.float32)
    dinv = (1.0 / np.sqrt(deg)).astype(np.float32)
    loops = np.arange(N, dtype=np.int64)
    es = np.concatenate([src, loops])
    ed = np.concatenate([dst, loops])
    ew = dinv[es] * dinv[ed]

    i1, w1, d1, T1, O1, n_et = _build_streams(es, ed, ew, C, NS, DT)
    maxTl = max(int(T1[:, 0].max()), 1)
    maxTh = max(int(T1[:, 1].max()), 1)
    maxTt = int((T1[:, 0] + T1[:, 1]).max())

    xT_f16 = np.ascontiguousarray(x.astype(np.float16).T)          # [F, N]
    xpT_f16 = np.ascontiguousarray(x[perm_np].astype(np.float16).T)  # [F, N]
    W_f16 = np.ascontiguousarray(W.astype(np.float16))
    b2 = np.ascontiguousarray(np.concatenate([b, b]).astype(np.float32))
    dwT = np.ascontiguousarray(disc_W.T.astype(np.float32))
    iota_np = np.tile(np.arange(_P, dtype=np.float16)[None, :], (_P, 1))
    rowmask_np = (np.arange(_P) < LAST).astype(np.float32)[:, None]

    # ---- device program -----------------------------------------------
    nc = bacc.Bacc("TRN2", target_bir_lowering=False, debug=False, num_devices=C)

    t_xT = nc.dram_tensor("xT16", [F, N], f16, kind="ExternalInput")
    t_xpT = nc.dram_tensor("xpT16", [F, N], f16, kind="ExternalInput")
    t_W = nc.dram_tensor("w16", [F, H], f16, kind="ExternalInput")
    t_b2 = nc.dram_tensor("b2vec", [H2], f32, kind="ExternalInput")
    t_a = nc.dram_tensor("avec", [1], f32, kind="ExternalInput")
    t_dwT = nc.dram_tensor("dwT", [H, H], f32, kind="ExternalInput")
    t_iota = nc.dram_tensor("iota", [_P, _P], f16, kind="ExternalInput")
    t_ident = nc.dram_tensor("ident_in", [_P, _P], f32, kind="ExternalInput")
    t_rowmask = nc.dram_tensor("rowmask", [_P, 1], f32, kind="ExternalInput")
    t_i1 = nc.dram_tensor("idx1", [_P, n_et * 8], mybir.dt.int16, kind="ExternalInput")
    t_w1 = nc.dram_tensor("wgt1", [_P, n_et], f16, kind="ExternalInput")
    t_d1 = nc.dram_tensor("dstl1", [_P, n_et], f16, kind="ExternalInput")

    t_out = nc.dram_tensor("pn_out", [_P, DT * 2], f32, kind="ExternalOutput")

    t_xw_lo = nc.dram_tensor("xw_lo", [_LO, H2], f16)
    t_xw_hi = nc.dram_tensor("xw_hi", [N - _LO, H2], f16)
    t_ar_in = nc.dram_tensor("ar_in", [H], f32)
    t_ar_out = nc.dram_tensor("ar_out", [H], f32, addr_space="Shared")

    CHUNK = 1024  # phase-1 node columns per load

    with tile.TileContext(nc) as tc:
        import contextlib

        ctx = contextlib.ExitStack()
        consts = ctx.enter_context(tc.tile_pool(name="consts", bufs=1))
        ph1 = ctx.enter_context(tc.tile_pool(name="ph1", bufs=2))
        ph1o = ctx.enter_context(tc.tile_pool(name="ph1o", bufs=3))
        ph1ps = ctx.enter_context(tc.tile_pool(name="ph1ps", bufs=2, space="PSUM"))
        glo = ctx.enter_context(tc.tile_pool(name="glo", bufs=2))
        ghi = ctx.enter_context(tc.tile_pool(name="ghi", bufs=2))
        stp = ctx.enter_context(tc.tile_pool(name="stp", bufs=2))
        aggps = ctx.enter_context(tc.tile_pool(name="aggps", bufs=3, space="PSUM"))
        misc = ctx.enter_context(tc.tile_pool(name="misc", bufs=2))
        miscps = ctx.enter_context(tc.tile_pool(name="miscps", bufs=1, space="PSUM"))

        # ---- constants ----
        W0 = consts.tile([_P, H], f16, tag="W0")
        W1 = consts.tile([_P, H], f16, tag="W1")
        nc.sync.dma_start(W0[:], t_W[0:_P, :])
        nc.sync.dma_start(W1[:], t_W[_P : 2 * _P, :])
        iota_t = consts.tile([_P, _P], f16, tag="iota")
        nc.sync.dma_start(iota_t[:], t_iota[:])
        b2_sb = consts.tile([1, H2], f32, tag="b2_sb")
        nc.sync.dma_start(b2_sb[:], t_b2[None, :])
        a_sb = consts.tile([1, 1], f32, tag="a_sb")
        nc.sync.dma_start(a_sb[:], t_a[None, :])
        dwT0 = consts.tile([_P, H], f32, tag="dwT0")
        dwT1 = consts.tile([_P, H], f32, tag="dwT1")
        nc.sync.dma_start(dwT0[:], t_dwT[0:_P, :])
        nc.sync.dma_start(dwT1[:], t_dwT[_P : 2 * _P, :])
        ones_row = consts.tile([1, _P], f32, tag="ones_row")
        nc.vector.memset(ones_row[:], 1.0)
        ones_col = consts.tile([_P, 1], f32, tag="ones_col")
        nc.vector.memset(ones_col[:], 1.0)

        # broadcasts via K=1 matmul
        bb_ps = miscps.tile([_P, H2], f32, tag="mps")
        nc.tensor.matmul(bb_ps[:], ones_row[:], b2_sb[:], start=True, stop=True)
        b_bc = consts.tile([_P, H2], f16, tag="b_bc")
        nc.vector.tensor_copy(b_bc[:], bb_ps[:])
        ab_ps = miscps.tile([_P, 1], f32, tag="mps")
        nc.tensor.matmul(ab_ps[:], ones_row[:], a_sb[:], start=True, stop=True)
        a_bc = consts.tile([_P, 1], f32, tag="a_bc")
        nc.vector.tensor_copy(a_bc[:], ab_ps[:])

        # ---- stream loads ----
        rowmask_sb = consts.tile([_P, 1], f32, tag="rowmask")
        nc.sync.dma_start(rowmask_sb[:], t_rowmask[:, :])
        i1_sb = consts.tile([_P, n_et * 8], mybir.dt.int16, tag="i1")
        w1_sb = consts.tile([_P, n_et], f16, tag="w1")
        d1_sb = consts.tile([_P, n_et], f16, tag="d1")
        nc.sync.dma_start(i1_sb[:], t_i1[:])
        nc.sync.dma_start(w1_sb[:], t_w1[:])
        nc.sync.dma_start(d1_sb[:], t_d1[:])

        # ---- phase 1: xw_cat = [x@W | x_p@W], lo rows then hi rows ----
        def phase1_range(r_start, r_end, t_dst):
            for r0 in range(r_start, r_end, CHUNK):
                cols = min(CHUNK, r_end - r0)
                xT0 = ph1.tile([_P, CHUNK], f16, tag="xT0")
                xT1 = ph1.tile([_P, CHUNK], f16, tag="xT1")
                xp0 = ph1.tile([_P, CHUNK], f16, tag="xp0")
                xp1 = ph1.tile([_P, CHUNK], f16, tag="xp1")
                nc.sync.dma_start(xT0[:, :cols], t_xT[0:_P, r0 : r0 + cols])
                nc.sync.dma_start(xT1[:, :cols], t_xT[_P : 2 * _P, r0 : r0 + cols])
                nc.sync.dma_start(xp0[:, :cols], t_xpT[0:_P, r0 : r0 + cols])
                nc.sync.dma_start(xp1[:, :cols], t_xpT[_P : 2 * _P, r0 : r0 + cols])
                for o in range(0, cols, _P):
                    m = min(_P, cols - o)
                    ps = ph1ps.tile([_P, H2], f32, tag="ph1ps")
                    nc.tensor.matmul(
                        ps[:m, 0:H], xT0[:, o : o + m], W0[:], start=True, stop=False
                    )
                    nc.tensor.matmul(
                        ps[:m, 0:H], xT1[:, o : o + m], W1[:], start=False, stop=True
                    )
                    nc.tensor.matmul(
                        ps[:m, H:H2], xp0[:, o : o + m], W0[:], start=True, stop=False
                    )
                    nc.tensor.matmul(
                        ps[:m, H:H2], xp1[:, o : o + m], W1[:], start=False, stop=True
                    )
                    xw_sb = ph1o.tile([_P, H2], f16, tag="xw_sb")
                    nc.any.tensor_copy(xw_sb[:m, :], ps[:m, :])
                    nc.sync.dma_start(
                        t_dst[r0 + o - r_start : r0 + o - r_start + m, :], xw_sb[:m, :]
                    )

        phase1_range(0, _LO, t_xw_lo)
        phase1_range(_LO, N, t_xw_hi)

        # ---- aggregation: one fused sweep over dst tiles ----
        zcat = consts.tile([_P, DT, H2], f16, tag="zcat")

        for dti in range(DT):
            Tl, Th = int(T1[dti, 0]), int(T1[dti, 1])
            Ol, Oh = int(O1[dti, 0]), int(O1[dti, 1])
            Tt = Tl + Th
            gl = gh = None
            if Tl:
                gl = glo.tile([_P, maxTl, H2], f16, tag="gl")
                nc.gpsimd.dma_gather(
                    gl[:, :Tl, :],
                    t_xw_lo[:, :],
                    i1_sb[:, 8 * Ol : 8 * (Ol + Tl)],
                    Tl * _P,
                    Tl * _P,
                    H2,
                    single_packet=(Tl * _P <= 1024),
                )
            if Th:
                gh = ghi.tile([_P, maxTh, H2], f16, tag="gh")
                nc.gpsimd.dma_gather(
                    gh[:, :Th, :],
                    t_xw_hi[:, :],
                    i1_sb[:, 8 * Oh : 8 * (Oh + Th)],
                    Th * _P,
                    Th * _P,
                    H2,
                    single_packet=(Th * _P <= 1024),
                )
            # batched weighted one-hot for all Tt edge-tiles of this dst tile
            eq = stp.tile([_P, maxTt, _P], f16, tag="eq")
            nc.vector.tensor_tensor(
                eq[:, :Tt, :],
                d1_sb[:, Ol : Ol + Tt, None].to_broadcast([_P, Tt, _P]),
                iota_t[:, None, :].to_broadcast([_P, Tt, _P]),
                mybir.AluOpType.is_equal,
            )
            stw = stp.tile([_P, maxTt, _P], f16, tag="stw")
            nc.vector.tensor_tensor(
                stw[:, :Tt, :],
                eq[:, :Tt, :],
                w1_sb[:, Ol : Ol + Tt, None].to_broadcast([_P, Tt, _P]),
                mybir.AluOpType.mult,
            )
            ps = aggps.tile([_P, H2], f32, tag="aggps")
            k = 0
            for j in range(Tl):
                nc.tensor.matmul(
                    ps[:], stw[:, j, :], gl[:, j, :], start=(k == 0), stop=(k == Tt - 1)
                )
                k += 1
            for j in range(Th):
                nc.tensor.matmul(
                    ps[:],
                    stw[:, Tl + j, :],
                    gh[:, j, :],
                    start=(k == 0),
                    stop=(k == Tt - 1),
                )
                k += 1
            nc.any.tensor_copy(zcat[:, dti, :], ps[:])

        # ---- batched bias + PReLU over zcat ----
        nc.vector.tensor_tensor(
            zcat[:, :, :],
            zcat[:, :, :],
            b_bc[:, None, :].to_broadcast([_P, DT, H2]),
            mybir.AluOpType.add,
        )
        PR = 8  # dst tiles per PReLU chunk
        for d0 in range(0, DT, PR):
            dn = min(PR, DT - d0)
            t1 = misc.tile([_P, PR, H2], f16, tag="t1")
            nc.vector.tensor_scalar(
                t1[:, :dn, :],
                zcat[:, d0 : d0 + dn, :],
                0.0,
                a_bc[:, 0:1],
                mybir.AluOpType.min,
                mybir.AluOpType.mult,
            )
            nc.vector.tensor_scalar(
                zcat[:, d0 : d0 + dn, :],
                zcat[:, d0 : d0 + dn, :],
                0.0,
                None,
                mybir.AluOpType.max,
            )
            nc.vector.tensor_tensor(
                zcat[:, d0 : d0 + dn, :],
                zcat[:, d0 : d0 + dn, :],
                t1[:, :dn, :],
                mybir.AluOpType.add,
            )
        # zero the invalid rows of the last dst tile (keeps summary exact)
        if LAST < _P:
            nc.vector.tensor_scalar(
                zcat[:, DT - 1, :],
                zcat[:, DT - 1, :],
                rowmask_sb[:, 0:1],
                None,
                mybir.AluOpType.mult,
            )

        # ---- summary: column sums of z1 over all nodes ----
        cacc = misc.tile([_P, H], f32, tag="cacc")
        nc.vector.reduce_sum(
            cacc[:],
            zcat[:, :, 0:H].rearrange("p d h -> p h d", d=DT, h=H),
            bass_rust.AxisListType.X,
        )
        cs_ps = miscps.tile([1, H], f32, tag="mps")
        nc.tensor.matmul(cs_ps[:], ones_col[:], cacc[:], start=True, stop=True)
        cs_sb = misc.tile([1, H], f32, tag="cs_sb")
        nc.vector.tensor_copy(cs_sb[:], cs_ps[:])
        nc.sync.dma_start(t_ar_in[None, :], cs_sb[:])
        nc.gpsimd.collective_compute(
            "AllReduce",
            mybir.AluOpType.add,
            replica_groups=[list(range(C))],
            ins=[t_ar_in[:]],
            outs=[t_ar_out[:]],
        )
        sums_sb = misc.tile([1, H], f32, tag="sums_sb")
        nc.sync.dma_start(sums_sb[:], t_ar_out[None, :])
        summ_sb = misc.tile([1, H], f32, tag="summ_sb")
        nc.scalar.activation(
            summ_sb[:], sums_sb[:], mybir.ActivationFunctionType.Sigmoid,
            scale=1.0 / N,
        )

        # ---- wsum = disc_W @ summary ----
        ident = consts.tile([_P, _P], f32, tag="ident")
        nc.sync.dma_start(ident[:], t_ident[:])
        sT = misc.tile([_P, 2], f32, tag="sT")
        for c_i in range(2):
            tp = miscps.tile([_P, _P], f32, tag="mps")
            nc.tensor.transpose(
                tp[:, 0:1],
                summ_sb[0:1, c_i * _P : (c_i + 1) * _P],
                ident[0:1, 0:1],
            )
            nc.vector.tensor_copy(sT[:, c_i : c_i + 1], tp[:, 0:1])
        ws_ps = miscps.tile([1, H], f32, tag="mps")
        nc.tensor.matmul(ws_ps[:], sT[:, 0:1], dwT0[:], start=True, stop=False)
        nc.tensor.matmul(ws_ps[:], sT[:, 1:2], dwT1[:], start=False, stop=True)
        ws2_sb = misc.tile([1, H2], f32, tag="ws2_sb")
        nc.vector.tensor_copy(ws2_sb[:, 0:H], ws_ps[:])
        nc.vector.tensor_copy(ws2_sb[:, H:H2], ws_ps[:])
        wb_ps = miscps.tile([_P, H2], f32, tag="mps")
        nc.tensor.matmul(wb_ps[:], ones_row[:], ws2_sb[:], start=True, stop=True)
        wsum_bc = consts.tile([_P, H2], f16, tag="wsum_bc")
        nc.vector.tensor_copy(wsum_bc[:], wb_ps[:])

        # ---- pos/neg dots: zcat *= [wsum|wsum]; reduce 256-chunks ----
        nc.vector.tensor_tensor(
            zcat[:, :, :],
            zcat[:, :, :],
            wsum_bc[:, None, :].to_broadcast([_P, DT, H2]),
            mybir.AluOpType.mult,
        )
        out_acc = misc.tile([_P, DT * 2], f32, tag="out_acc")
        nc.vector.reduce_sum(
            out_acc[:],
            zcat[:, :, :].rearrange("p d (t h) -> p (d t) h", t=2, h=H),
            bass_rust.AxisListType.X,
        )
        nc.sync.dma_start(t_out[:], out_acc[:])
        ctx.close()

    nc.compile()

    in_maps = []
    for c in range(C):
        in_maps.append(
            {
                "xT16": xT_f16,
                "xpT16": xpT_f16,
                "w16": W_f16,
                "b2vec": b2,
                "avec": a,
                "dwT": dwT,
                "iota": iota_np,
                "ident_in": np.eye(_P, dtype=np.float32),
                "rowmask": rowmask_np,
                "idx1": i1[c],
                "wgt1": w1[c],
                "dstl1": d1[c],
            }
        )

    if os.environ.get("KERNEL_SIM", "0") == "1":
        from concourse import bass_interp

        sim = bass_interp.MultiCoreSim(nc, C)
        for c in range(C):
            for k, v in in_maps[c].items():
                sim.cores[c].tensor(k)[:] = v
        sim.simulate()
        results = [
            {"pn_out": np.array(sim.cores[c].tensor("pn_out"))} for c in range(C)
        ]
    else:
        trace = os.environ.get("KERNEL_TRACE", "0") == "1"
        kw = {}
        if trace:
            kw["trace"] = True
        res = run_bass_kernel_spmd(nc, in_maps, core_ids=list(range(C)), **kw)
        kernel.last_result = res
        results = res.results

    pos = np.zeros(N, np.float32)
    neg = np.zeros(N, np.float32)
    for c in range(C):
        arr = results[c]["pn_out"].reshape(_P, DT, 2)
        pos[c * NS : (c + 1) * NS] = arr[:, :, 0].T.reshape(-1)[:NS]
        neg[c * NS : (c + 1) * NS] = arr[:, :, 1].T.reshape(-1)[:NS]
    return pos, neg
